# revision 1
# baseline (speedup 1.0000x reference)
"""Trainium2 Bass kernel for nn_BilateralModule (bilateral filter + Canny
NMS + hysteresis), data-parallel across 8 NeuronCores (2 images per core).

This environment charges ~40-90us for every unique instruction per execution
(instruction streaming), while loop-resident instructions cost ~1.5us plus
engine time. The whole pipeline is therefore expressed as runtime For_i loops
with dynamically sliced access patterns (small static code, big dynamic
work): an image loop, 9x9 bilateral tap loops over the full window (invalid
taps get r2=1e9 from a host-supplied table so exp() yields weight 0; the
center tap lands on weight 1 automatically), an NMS channel loop, and a
hysteresis iteration loop.

Also includes a workaround for this walrus build accepting at most ONE
sync-wait per instruction: extra waits are hoisted onto NoOps inserted just
before the instruction on the same engine (identical program-order
semantics).
"""
import numpy as np

import concourse.bass as bass
import concourse.bacc as bacc
import concourse.mybir as mybir
from concourse.mybir import AluOpType as A, ActivationFunctionType as F
from concourse.tile import TileContext

F32 = mybir.dt.float32
F16 = mybir.dt.float16
U8 = mybir.dt.uint8

H = W = 512
PAD = 4
WP = W + 2 * PAD  # 520
WH = W + 2  # 514
MAGIC = 12582912.0
GC = -0.5 / 75.0 ** 2
HIGH_T = 150.0
LOW_T = 50.0
T22 = float(np.tan(np.radians(22.5)))
T67 = float(np.tan(np.radians(67.5)))
HYST_ITERS = 4
NB = 2
NCORES = 8


def host_r2_table():
    t = np.zeros((9, 9), np.float32)
    for iy in range(9):
        for ix in range(9):
            r2 = (iy - 4) ** 2 + (ix - 4) ** 2
            t[iy, ix] = float(r2) if r2 <= 16 else 1.0e9
    return np.broadcast_to(t, (128, 9, 9)).copy()


def build(rep=1):
    nc = bacc.Bacc()
    xp = nc.dram_tensor("xp", [NB, 3, WP, WP], F32, kind="ExternalInput")
    r2d = nc.dram_tensor("r2t", [128, 9, 9], F32, kind="ExternalInput")
    out = nc.dram_tensor("edges", [NB, H, W], F32, kind="ExternalOutput")
    v = nc.vector
    s = nc.scalar
    g = nc.gpsimd

    with TileContext(nc) as tc:
        with tc.tile_pool(name="glob", bufs=1) as pg:
            r2t = pg.tile([128, 9, 9], F32, tag="r2t", name="r2t")
            nc.sync.dma_start(out=r2t[:, :, :], in_=r2d[:, :, :])

            filt = pg.tile([128, 3, 4, W], F32, tag="filt", name="filt")
            nmst = pg.tile([128, 4, W], F32, tag="nmst", name="nmst")

            with tc.For_i(0, rep, 1) as _r:
              with tc.For_i(0, NB, 1) as b:
                  pimg_cm = tc.tile_pool(name="pimg", bufs=1)
                  pimg = pimg_cm.__enter__()
                  imgA = pimg.tile([128, 3, 12, WP], F16, tag="imgA", name="imgA")
                  num = pimg.tile([128, 3, 4, W], F32, tag="num", name="num")
                  den = pimg.tile([128, 4, W], F32, tag="den", name="den")
                  # ---------- load + quantize + halo assemble ----------
                  with tc.tile_pool(name="pq", bufs=1) as pq:
                      qs = pq.tile([128, 3, 4, WP], F32, tag="qs", name="qs")
                      qe = pq.tile([2, 3, 4, WP], F32, tag="qe", name="qe")
                      b16 = pq.tile([128, 3, 4, WP], F16, tag="b16", name="b16")
                      e16 = pq.tile([2, 3, 4, WP], F16, tag="e16", name="e16")
                      nc.sync.dma_start(
                          out=qs[:, :, :, :],
                          in_=xp[bass.ds(b, 1), :, 0:H, :]
                          .rearrange("o c (p r) x -> (o p) c r x", r=4),
                      )
                      nc.sync.dma_start(
                          out=qe[:, :, :, :],
                          in_=xp[bass.ds(b, 1), :, H : H + 8, :]
                          .rearrange("o c (p r) x -> (o p) c r x", r=4),
                      )
                      for tin, tout in ((qs, b16), (qe, e16)):
                          v.tensor_scalar(tin[:, :, :, :], tin[:, :, :, :], 0.0, 1.0, A.max, A.min)
                          v.tensor_scalar(tin[:, :, :, :], tin[:, :, :, :], 255.0, MAGIC, A.mult, A.add)
                          v.tensor_scalar(tout[:, :, :, :], tin[:, :, :, :], MAGIC, None, A.subtract)
                      nc.sync.dma_start(out=imgA[:, :, 0:4, :], in_=b16[:, :, :, :])
                      nc.sync.dma_start(out=imgA[0:127, :, 4:8, :], in_=b16[1:128, :, :, :])
                      nc.sync.dma_start(out=imgA[127:128, :, 4:8, :], in_=e16[0:1, :, :, :])
                      nc.sync.dma_start(out=imgA[0:126, :, 8:12, :], in_=b16[2:128, :, :, :])
                      nc.sync.dma_start(out=imgA[126:127, :, 8:12, :], in_=e16[0:1, :, :, :])
                      nc.sync.dma_start(out=imgA[127:128, :, 8:12, :], in_=e16[1:2, :, :, :])

                  # ---------- bilateral taps ----------
                  v.memset(num[:, :, :, :], 0.0)
                  g.memset(den[:, :, :], 0.0)
                  ctr = imgA[:, :, 4:8, 4 : 4 + W]
                  with tc.tile_pool(name="pt", bufs=2) as pt:
                      dt_ = pt.tile([128, 3, 4, W], F16, tag="dt", name="dt")
                      cd = pt.tile([128, 4, W], F16, tag="cd", name="cd")
                      wt = pt.tile([128, 4, W], F32, tag="wt", name="wt")
                      pr = pt.tile([128, 3, 4, W], F32, tag="pr", name="pr")
                      with tc.For_i(0, 9, 1) as iy:
                          with tc.For_i(0, 9, 1) as ix:
                              sh = imgA[:, :, bass.ds(iy, 4), bass.ds(ix, W)]
                              v.tensor_tensor(out=dt_[:, :, :, :], in0=sh, in1=ctr, op=A.subtract)
                              s.activation(out=dt_[:, :, :, :], in_=dt_[:, :, :, :], func=F.Abs)
                              v.tensor_tensor(out=cd[:, :, :], in0=dt_[:, 0, :, :], in1=dt_[:, 1, :, :], op=A.add)
                              v.tensor_tensor(out=cd[:, :, :], in0=cd[:, :, :], in1=dt_[:, 2, :, :], op=A.add)
                              s.activation(out=wt[:, :, :], in_=cd[:, :, :], func=F.Square)
                              r2bc = (
                                  r2t[:, bass.ds(iy, 1), bass.ds(ix, 1)]
                                  .rearrange("p a z -> p (a z)")
                                  .unsqueeze(1)
                                  .broadcast_to([128, 4, W])
                              )
                              v.tensor_tensor(out=wt[:, :, :], in0=wt[:, :, :], in1=r2bc, op=A.add)
                              s.activation(out=wt[:, :, :], in_=wt[:, :, :], func=F.Exp, scale=GC)
                              wb = wt[:, :, :].unsqueeze(1).broadcast_to([128, 3, 4, W])
                              v.tensor_tensor(out=pr[:, :, :, :], in0=wb, in1=sh, op=A.mult)
                              v.tensor_tensor(out=num[:, :, :, :], in0=num[:, :, :, :], in1=pr[:, :, :, :], op=A.add)
                              g.tensor_tensor(out=den[:, :, :], in0=den[:, :, :], in1=wt[:, :, :], op=A.add)

                  # ---------- divide ----------
                  with tc.tile_pool(name="pdv", bufs=1) as pdv:
                      rd = pdv.tile([128, 4, W], F32, tag="rd", name="rd")
                      v.reciprocal(out=rd[:, :, :], in_=den[:, :, :])
                      rdb = rd[:, :, :].unsqueeze(1).broadcast_to([128, 3, 4, W])
                      v.tensor_tensor(out=filt[:, :, :, :], in0=num[:, :, :, :], in1=rdb, op=A.mult)
                  pimg_cm.__exit__(None, None, None)

                  # ---------- NMS gradients (channel loop) ----------
                  with tc.tile_pool(name="psl", bufs=1) as psl:
                      gxs = psl.tile([128, 4, W], F32, tag="gxs", name="gxs")
                      gys = psl.tile([128, 4, W], F32, tag="gys", name="gys")
                      mags = psl.tile([128, 4, W], F32, tag="mags", name="mags")
                      with tc.tile_pool(name="pn", bufs=1) as pn:
                          gx3 = pn.tile([128, 3, 4, W], F32, tag="gx3", name="gx3")
                          gy3 = pn.tile([128, 3, 4, W], F32, tag="gy3", name="gy3")
                          mag3 = pn.tile([128, 3, 4, W], F32, tag="mag3", name="mag3")
                          with tc.tile_pool(name="pnh", bufs=1) as pnh:
                              fh = pnh.tile([128, 6, WH], F32, tag="fh", name="fh")
                              syt = pnh.tile([128, 4, WH], F32, tag="syt", name="syt")
                              sxh = pnh.tile([128, 6, W], F32, tag="sxh", name="sxh")
                              axt = pnh.tile([128, 4, W], F32, tag="axt", name="axt")
                              ayt = pnh.tile([128, 4, W], F32, tag="ayt", name="ayt")
                              for c in range(3):
                                  f = filt[:, bass.ds(c, 1), :, :].rearrange("p o r x -> p (o r) x")
                                  nc.sync.dma_start(out=fh[:, 1:5, 1 : 1 + W], in_=f[:, :, :])
                                  nc.sync.dma_start(out=fh[1:128, 0:1, 1 : 1 + W], in_=f[0:127, 3:4, :])
                                  nc.sync.dma_start(out=fh[0:1, 0:1, 1 : 1 + W], in_=f[0:1, 0:1, :])
                                  nc.sync.dma_start(out=fh[0:127, 5:6, 1 : 1 + W], in_=f[1:128, 0:1, :])
                                  nc.sync.dma_start(out=fh[127:128, 5:6, 1 : 1 + W], in_=f[127:128, 3:4, :])
                                  nc.sync.dma_start(out=fh[:, :, 0:1], in_=fh[:, :, 1:2])
                                  nc.sync.dma_start(out=fh[:, :, WH - 1 : WH], in_=fh[:, :, WH - 2 : WH - 1])
                                  gxc = gx3[:, bass.ds(c, 1), :, :].rearrange("p o r x -> p (o r) x")
                                  gyc = gy3[:, bass.ds(c, 1), :, :].rearrange("p o r x -> p (o r) x")
                                  mgc = mag3[:, bass.ds(c, 1), :, :].rearrange("p o r x -> p (o r) x")
                                  v.scalar_tensor_tensor(out=syt[:, :, :], in0=fh[:, 1:5, :], scalar=2.0, in1=fh[:, 0:4, :], op0=A.mult, op1=A.add)
                                  v.tensor_tensor(out=syt[:, :, :], in0=syt[:, :, :], in1=fh[:, 2:6, :], op=A.add)
                                  v.tensor_tensor(out=gxc, in0=syt[:, :, 2:WH], in1=syt[:, :, 0:W], op=A.subtract)
                                  v.scalar_tensor_tensor(out=sxh[:, :, :], in0=fh[:, :, 1 : 1 + W], scalar=2.0, in1=fh[:, :, 0:W], op0=A.mult, op1=A.add)
                                  v.tensor_tensor(out=sxh[:, :, :], in0=sxh[:, :, :], in1=fh[:, :, 2 : 2 + W], op=A.add)
                                  v.tensor_tensor(out=gyc, in0=sxh[:, 2:6, :], in1=sxh[:, 0:4, :], op=A.subtract)
                                  s.activation(out=axt[:, :, :], in_=gxc, func=F.Abs)
                                  s.activation(out=ayt[:, :, :], in_=gyc, func=F.Abs)
                                  v.tensor_tensor(out=mgc, in0=axt[:, :, :], in1=ayt[:, :, :], op=A.add)

                          # ---------- channel select ----------
                          m12 = pn.tile([128, 4, W], F32, tag="m12", name="m12")
                          c0 = pn.tile([128, 4, W], U8, tag="c0", name="c0")
                          c12 = pn.tile([128, 4, W], U8, tag="c12", name="c12")
                          v.tensor_tensor(out=m12[:, :, :], in0=mag3[:, 1, :, :], in1=mag3[:, 2, :, :], op=A.max)
                          v.tensor_tensor(out=c0[:, :, :], in0=mag3[:, 0, :, :], in1=m12[:, :, :], op=A.is_ge)
                          v.tensor_tensor(out=c12[:, :, :], in0=mag3[:, 1, :, :], in1=mag3[:, 2, :, :], op=A.is_ge)
                          for sel, ch in ((gxs, gx3), (gys, gy3), (mags, mag3)):
                              v.tensor_copy(out=sel[:, :, :], in_=ch[:, 2, :, :])
                              v.copy_predicated(out=sel[:, :, :], mask=c12[:, :, :], data=ch[:, 1, :, :])
                              v.copy_predicated(out=sel[:, :, :], mask=c0[:, :, :], data=ch[:, 0, :, :])

                      # ---------- suppress ----------
                      with tc.tile_pool(name="psp", bufs=1) as psp:
                          mh = psp.tile([128, 6, WH], F32, tag="mh", name="mh")
                          v.memset(mh[:, :, :], 0.0)
                          nc.sync.dma_start(out=mh[:, 1:5, 1 : 1 + W], in_=mags[:, :, :])
                          nc.sync.dma_start(out=mh[1:128, 0:1, 1 : 1 + W], in_=mags[0:127, 3:4, :])
                          nc.sync.dma_start(out=mh[0:127, 5:6, 1 : 1 + W], in_=mags[1:128, 0:1, :])
                          axs = psp.tile([128, 4, W], F32, tag="axs", name="axs")
                          sg = psp.tile([128, 4, W], F32, tag="sg", name="sg")
                          u = psp.tile([128, 4, W], F32, tag="u", name="u")
                          s.activation(out=axs[:, :, :], in_=gxs[:, :, :], func=F.Abs)
                          s.activation(out=sg[:, :, :], in_=gxs[:, :, :], func=F.Sign)
                          v.tensor_tensor(out=u[:, :, :], in0=gys[:, :, :], in1=sg[:, :, :], op=A.mult)
                          tA = psp.tile([128, 4, W], F32, tag="tA", name="tA")
                          TA = psp.tile([128, 4, W], F32, tag="TA", name="TA")
                          ntA = psp.tile([128, 4, W], F32, tag="ntA", name="ntA")
                          nTA = psp.tile([128, 4, W], F32, tag="nTA", name="nTA")
                          v.tensor_scalar(tA[:, :, :], axs[:, :, :], T22, None, A.mult)
                          v.tensor_scalar(TA[:, :, :], axs[:, :, :], T67, None, A.mult)
                          v.tensor_scalar(ntA[:, :, :], tA[:, :, :], -1.0, None, A.mult)
                          v.tensor_scalar(nTA[:, :, :], TA[:, :, :], -1.0, None, A.mult)
                          b1 = psp.tile([128, 4, W], U8, tag="b1", name="b1")
                          b2 = psp.tile([128, 4, W], U8, tag="b2", name="b2")
                          d0m = psp.tile([128, 4, W], U8, tag="d0m", name="d0m")
                          d45 = psp.tile([128, 4, W], U8, tag="d45", name="d45")
                          d90 = psp.tile([128, 4, W], U8, tag="d90", name="d90")
                          zm = psp.tile([128, 4, W], U8, tag="zm", name="zm")
                          v.tensor_tensor(out=b1[:, :, :], in0=u[:, :, :], in1=ntA[:, :, :], op=A.is_ge)
                          v.tensor_tensor(out=b2[:, :, :], in0=u[:, :, :], in1=tA[:, :, :], op=A.is_lt)
                          v.tensor_tensor(out=d0m[:, :, :], in0=b1[:, :, :], in1=b2[:, :, :], op=A.logical_and)
                          v.tensor_scalar(zm[:, :, :], mags[:, :, :], 0.0, None, A.is_equal)
                          v.tensor_tensor(out=d0m[:, :, :], in0=d0m[:, :, :], in1=zm[:, :, :], op=A.logical_or)
                          v.tensor_tensor(out=b1[:, :, :], in0=u[:, :, :], in1=tA[:, :, :], op=A.is_ge)
                          v.tensor_tensor(out=b2[:, :, :], in0=u[:, :, :], in1=TA[:, :, :], op=A.is_lt)
                          v.tensor_tensor(out=d45[:, :, :], in0=b1[:, :, :], in1=b2[:, :, :], op=A.logical_and)
                          v.tensor_tensor(out=b1[:, :, :], in0=u[:, :, :], in1=TA[:, :, :], op=A.is_ge)
                          v.tensor_tensor(out=b2[:, :, :], in0=u[:, :, :], in1=nTA[:, :, :], op=A.is_lt)
                          v.tensor_tensor(out=d90[:, :, :], in0=b1[:, :, :], in1=b2[:, :, :], op=A.logical_or)
                          n1 = psp.tile([128, 4, W], F32, tag="n1", name="n1")
                          n2 = psp.tile([128, 4, W], F32, tag="n2", name="n2")

                          def nbr(dy, dx):
                              return mh[:, 1 + dy : 5 + dy, 1 + dx : 1 + dx + W]

                          v.tensor_copy(out=n1[:, :, :], in_=nbr(-1, -1))
                          v.copy_predicated(out=n1[:, :, :], mask=d90[:, :, :], data=nbr(-1, 0))
                          v.copy_predicated(out=n1[:, :, :], mask=d45[:, :, :], data=nbr(-1, 1))
                          v.copy_predicated(out=n1[:, :, :], mask=d0m[:, :, :], data=nbr(0, 1))
                          v.tensor_copy(out=n2[:, :, :], in_=nbr(1, 1))
                          v.copy_predicated(out=n2[:, :, :], mask=d90[:, :, :], data=nbr(1, 0))
                          v.copy_predicated(out=n2[:, :, :], mask=d45[:, :, :], data=nbr(1, -1))
                          v.copy_predicated(out=n2[:, :, :], mask=d0m[:, :, :], data=nbr(0, -1))
                          k1 = psp.tile([128, 4, W], U8, tag="k1", name="k1")
                          k2 = psp.tile([128, 4, W], U8, tag="k2", name="k2")
                          v.tensor_tensor(out=k1[:, :, :], in0=mags[:, :, :], in1=n1[:, :, :], op=A.is_ge)
                          v.tensor_tensor(out=k2[:, :, :], in0=mags[:, :, :], in1=n2[:, :, :], op=A.is_ge)
                          v.tensor_tensor(out=k1[:, :, :], in0=k1[:, :, :], in1=k2[:, :, :], op=A.logical_and)
                          v.memset(nmst[:, :, :], 0.0)
                          v.copy_predicated(out=nmst[:, :, :], mask=k1[:, :, :], data=mags[:, :, :])

                  # ---------- hysteresis ----------
                  with tc.tile_pool(name="phy", bufs=1) as phy:
                      st = phy.tile([128, 4, WH], F16, tag="st", name="st")
                      sc = phy.tile([128, 4, WH], F16, tag="sc", name="sc")
                      wk = phy.tile([128, 4, WH], F16, tag="wk", name="wk")
                      hdil = phy.tile([128, 4, WH], F16, tag="hdil", name="hdil")
                      vdil = phy.tile([128, 6, WH], F16, tag="vdil", name="vdil")
                      dil = phy.tile([128, 4, WH], F16, tag="dil", name="dil")
                      v.memset(st[:, :, :], 0.0)
                      v.memset(sc[:, :, :], 0.0)
                      v.memset(wk[:, :, :], 0.0)
                      v.memset(vdil[:, :, :], 0.0)
                      v.tensor_scalar(st[:, :, 1 : 1 + W], nmst[:, :, :], HIGH_T, None, A.is_gt)
                      v.tensor_scalar(wk[:, :, 1 : 1 + W], nmst[:, :, :], LOW_T, None, A.is_gt)
                      st2d = st[:, :, :].rearrange("p a x -> p (a x)")
                      sc2d = sc[:, :, :].rearrange("p a x -> p (a x)")
                      wk2d = wk[:, :, :].rearrange("p a x -> p (a x)")
                      with tc.For_i(0, HYST_ITERS, 1) as it:
                          v.tensor_tensor_scan(out=sc2d, data0=wk2d, data1=st2d, initial=0.0, op0=A.mult, op1=A.max)
                          v.tensor_tensor_scan(out=st2d[:, ::-1], data0=wk2d[:, ::-1], data1=sc2d[:, ::-1], initial=0.0, op0=A.mult, op1=A.max)
                          v.tensor_tensor(out=hdil[:, :, 1 : 1 + W], in0=st[:, :, 0:W], in1=st[:, :, 2 : 2 + W], op=A.max)
                          v.tensor_tensor(out=vdil[:, 1:5, 1 : 1 + W], in0=hdil[:, :, 1 : 1 + W], in1=st[:, :, 1 : 1 + W], op=A.max)
                          nc.sync.dma_start(out=vdil[1:128, 0:1, 1 : 1 + W], in_=vdil[0:127, 4:5, 1 : 1 + W])
                          nc.sync.dma_start(out=vdil[0:127, 5:6, 1 : 1 + W], in_=vdil[1:128, 1:2, 1 : 1 + W])
                          v.tensor_tensor(out=dil[:, :, :], in0=vdil[:, 0:4, :], in1=vdil[:, 2:6, :], op=A.max)
                          v.tensor_tensor(out=dil[:, :, :], in0=dil[:, :, :], in1=vdil[:, 1:5, :], op=A.max)
                          v.tensor_tensor(out=dil[:, :, :], in0=dil[:, :, :], in1=wk[:, :, :], op=A.mult)
                          v.tensor_tensor(out=st[:, :, :], in0=st[:, :, :], in1=dil[:, :, :], op=A.max)
                      o32 = phy.tile([128, 4, W], F32, tag="o32", name="o32")
                      v.tensor_copy(out=o32[:, :, :], in_=st[:, :, 1 : 1 + W])
                      nc.sync.dma_start(
                          out=out[bass.ds(b, 1), :, :].rearrange("o (p r) x -> (o p) r x", r=4),
                          in_=o32[:, :, :],
                      )
    nc.finalize()
    return nc


# ---------------------------------------------------------------------------
# walrus 1-sync-wait-per-instruction workaround (BIR JSON post-pass)
# ---------------------------------------------------------------------------
import json as _json

_ws_counter = [0]


def _split_instruction_list(instrs):
    out = []
    for ins in instrs:
        si = ins.get("sync_info")
        waits = (si or {}).get("on_wait") or []
        if len(waits) > 1:
            for wcond in waits[:-1]:
                _ws_counter[0] += 1
                out.append({
                    "debug": ins.get("debug", 0),
                    "engine": ins["engine"],
                    "ins": [],
                    "name": f"I-waitsplit-{_ws_counter[0]}",
                    "opcode": "NoOp",
                    "outs": [],
                    "sync_info": {"on_wait": [wcond], "on_update": []},
                })
            si = dict(si)
            si["on_wait"] = [waits[-1]]
            ins = dict(ins)
            ins["sync_info"] = si
        out.append(ins)
    return out


def _walk_split(obj):
    if isinstance(obj, dict):
        for k, val in obj.items():
            if k == "instructions" and isinstance(val, list):
                obj[k] = _split_instruction_list(val)
            else:
                _walk_split(val)
    elif isinstance(obj, list):
        for val in obj:
            _walk_split(val)


def _split_multiwait_bir(bir_json):
    j = _json.loads(bir_json)
    _walk_split(j)
    return _json.dumps(j).encode()


_patched = [False]


def _install_bir_patch():
    if _patched[0]:
        return
    _patched[0] = True
    import concourse.bass_utils as bu

    orig = bu.compile_bir_kernel

    def patched(bir_json, tmpdir, neff_name="file.neff"):
        return orig(_split_multiwait_bir(bir_json), tmpdir, neff_name)

    bu.compile_bir_kernel = patched
    try:
        import concourse.bass2jax as b2j

        b2j.compile_bir_kernel = patched
    except Exception:
        pass


# ---------------------------------------------------------------------------
# host entry point
# ---------------------------------------------------------------------------
_cache = {}


def _get_program(rep=1):
    key = ("nc", rep)
    if key not in _cache:
        _install_bir_patch()
        _cache[key] = build(rep=rep)
    return _cache[key]


def kernel(x):
    """x: [16,3,512,512] float32 -> edges [16,1,512,512] float32."""
    from concourse.bass_utils import run_bass_kernel_spmd

    x = np.asarray(x, dtype=np.float32)
    B = x.shape[0]
    assert x.shape == (NCORES * NB, 3, H, W), x.shape
    nc = _get_program()
    xpad = np.pad(x, ((0, 0), (0, 0), (PAD, PAD), (PAD, PAD)), mode="reflect")
    r2t = host_r2_table()
    in_maps = [
        {"xp": np.ascontiguousarray(xpad[i * NB : (i + 1) * NB]), "r2t": r2t}
        for i in range(NCORES)
    ]
    res = run_bass_kernel_spmd(nc, in_maps, core_ids=list(range(NCORES)))
    out = np.empty((B, 1, H, W), np.float32)
    for i in range(NCORES):
        out[i * NB : (i + 1) * NB, 0] = res.results[i]["edges"]
    return out



# revision 21
# speedup vs baseline: 6.5569x; 6.5569x over previous
"""Trainium2 Bass kernel for nn_BilateralModule (bilateral filter + Canny
NMS + hysteresis), data-parallel across 8 NeuronCores (2 images per core).

Fully unrolled design (no runtime For_i loops except the benchmark rep
loop): the terminal charges ~engine-time + small per-instruction overhead
for straight-line code, while For_i iterations carry a large per-iteration
sync storm (drains + semaphores on all five engines). The bilateral uses
the exact 49-tap circular window with static spatial weights folded into
per-tap immediates, f16 compute in the DVE 4x path (TensorScalarPtr ops),
and splits work across DVE (diff/square/products/accumulate), Act
(|.|, exp) and Pool (channel sums, den accumulate).

Layout: partition p holds image rows 4p..4p+3; dim1 fuses (channel, image)
c*NB+b so engine views stay within 3 free dims.

Also includes a workaround for this walrus build accepting at most ONE
sync-wait per instruction: extra waits are hoisted onto NoOps inserted just
before the instruction on the same engine (identical program-order
semantics).
"""
import numpy as np

import concourse.bass as bass
import concourse.bacc as bacc
import concourse.mybir as mybir
from concourse.mybir import AluOpType as A, ActivationFunctionType as F
from concourse.tile import TileContext

F32 = mybir.dt.float32
F16 = mybir.dt.float16
U8 = mybir.dt.uint8

H = W = 512
PAD = 4
WP = W + 2 * PAD  # 520
WH = W + 2  # 514
MAGIC = 12582912.0
GC = -0.5 / 75.0 ** 2
CS = 32.0  # |diff| pre-scale so cd^2 stays in f16 range
GCS = GC * CS * CS
HIGH_T = 150.0
LOW_T = 50.0
T22 = float(np.tan(np.radians(22.5)))
T67 = float(np.tan(np.radians(67.5)))
HYST_ITERS = 4
NB = 2
NCORES = 8
C6 = 3 * NB

TAPS = [
    (dy, dx)
    for dy in range(-PAD, PAD + 1)
    for dx in range(-PAD, PAD + 1)
    if 0 < dy * dy + dx * dx <= PAD * PAD
]


def build(rep=1):
    nc = bacc.Bacc()
    xp = nc.dram_tensor("xp", [NB, 3, WP, WP], F32, kind="ExternalInput")
    out = nc.dram_tensor("edges", [NB, H, W], F32, kind="ExternalOutput")
    v = nc.vector
    s = nc.scalar
    g = nc.gpsimd

    def stt(eng, out, in0, in1, op0, op1, scalar=0.0):
        eng.scalar_tensor_tensor(out=out, in0=in0, scalar=scalar, in1=in1,
                                 op0=op0, op1=op1)

    # activation float biases require pre-registered const APs
    for r2v in sorted({dy * dy + dx * dx for dy, dx in TAPS}):
        val = float(GC * r2v)
        t = nc.alloc_sbuf_tensor(f"const-bias-{r2v}", [128, 1], F32)
        nc.gpsimd.memset(t.ap(), val)
        nc.const_aps.aps[(F32, val)] = t.ap()

    with TileContext(nc) as tc:
        with tc.For_i(0, rep, 1) as _r:
            # pAcc: den lives through NMS (its buffer is reused as nmst)
            pacc_cm = tc.tile_pool(name="pacc", bufs=1)
            pacc = pacc_cm.__enter__()
            den = pacc.tile([128, NB, 4, W], F32, tag="den", name="den")
            nmst = den  # buffer reuse: den is dead once rcp is computed

            pnum_cm = tc.tile_pool(name="pnum", bufs=1)
            pnum = pnum_cm.__enter__()
            num = pnum.tile([128, C6, 4, W], F32, tag="num", name="num")

            pimg_cm = tc.tile_pool(name="pimg", bufs=1)
            pimg = pimg_cm.__enter__()
            imgA = pimg.tile([128, C6, 12, WP], F16, tag="imgA", name="imgA")

            # ---------- load + quantize straight into imgA rows 0:4 ----
            with tc.tile_pool(name="pq", bufs=1) as pq:
                qs = pq.tile([128, NB, 3, 2, WP], F32, tag="qs", name="qs")
                qe = pq.tile([8, NB, 3, 1, WP], F32, tag="qe", name="qe")
                e16 = pq.tile([8, C6, 1, WP], F16, tag="e16", name="e16")
                src = xp[:, :, 0:H, :].rearrange("b c (p r) x -> p b c r x", r=4)
                for h0 in (0, 2):
                    nc.sync.dma_start(out=qs[:, :, :, :, :], in_=src[:, :, :, h0 : h0 + 2, :])
                    v.tensor_scalar(qs[:, :, :, :, :], qs[:, :, :, :, :], 0.0, 1.0, A.max, A.min)
                    v.tensor_scalar(qs[:, :, :, :, :], qs[:, :, :, :, :], 255.0, MAGIC, A.mult, A.add)
                    dst = imgA[:, :, h0 : h0 + 2, :].rearrange("p (b c) r x -> p b c r x", c=3)
                    v.tensor_scalar(dst, qs[:, :, :, :, :], MAGIC, None, A.subtract)
                nc.sync.dma_start(
                    out=qe[:, :, :, :, :],
                    in_=xp[:, :, H : H + 8, :].rearrange("b c (p r) x -> p b c r x", r=1),
                )
                v.tensor_scalar(qe[:, :, :, :, :], qe[:, :, :, :, :], 0.0, 1.0, A.max, A.min)
                v.tensor_scalar(qe[:, :, :, :, :], qe[:, :, :, :, :], 255.0, MAGIC, A.mult, A.add)
                e16v = e16[:, :, :, :].rearrange("p (b c) r x -> p b c r x", c=3)
                v.tensor_scalar(e16v, qe[:, :, :, :, :], MAGIC, None, A.subtract)
                nc.sync.dma_start(out=imgA[0:127, :, 4:8, :], in_=imgA[1:128, :, 0:4, :])
                nc.sync.dma_start(out=imgA[0:126, :, 8:12, :], in_=imgA[2:128, :, 0:4, :])
                for r in range(4):
                    nc.sync.dma_start(out=imgA[127:128, :, 4 + r, :], in_=e16[r : r + 1, :, 0, :])
                    nc.sync.dma_start(out=imgA[126:127, :, 8 + r, :], in_=e16[r : r + 1, :, 0, :])
                    nc.sync.dma_start(out=imgA[127:128, :, 8 + r, :], in_=e16[4 + r : 5 + r, :, 0, :])

            # ---------- bilateral: 48 unrolled taps + center ----------
            # f16 only where values are exact integers (image, |diff|, channel
            # sums <= 765); everything from Square onward is f32.
            ctr6 = imgA[:, :, 4:8, 4 : 4 + W]
            v.tensor_tensor(out=num[:, :, :, :], in0=ctr6, in1=ctr6, op=A.max)
            g.memset(den[:, :, :, :], 1.0)
            with tc.tile_pool(name="pt", bufs=1) as pt:
                dt_ = pt.tile([128, C6, 4, W], F16, tag="dt", name="dt")
                cds = pt.tile([128, NB, 4, W], F16, tag="cds", name="cds")
                sqw = pt.tile([128, NB, 4, W], F32, tag="sqw", name="sqw")
                pr = pt.tile([128, NB, 4, W], F32, tag="pr", name="pr")
                for dy, dx in TAPS:
                    sy, sx = dy + PAD, dx + PAD
                    bias_r2 = float(GC * (dy * dy + dx * dx))
                    sh6 = imgA[:, :, sy : sy + 4, sx : sx + W]
                    v.tensor_tensor(out=dt_[:, :, :, :], in0=sh6, in1=ctr6, op=A.subtract)
                    s.activation(out=dt_[:, :, :, :], in_=dt_[:, :, :, :], func=F.Abs)
                    g.tensor_tensor(out=cds[:, :, :, :], in0=dt_[:, 0::3, :, :],
                                    in1=dt_[:, 1::3, :, :], op=A.add)
                    g.tensor_tensor(out=cds[:, :, :, :], in0=cds[:, :, :, :],
                                    in1=dt_[:, 2::3, :, :], op=A.add)
                    s.activation(out=sqw[:, :, :, :], in_=cds[:, :, :, :], func=F.Square)
                    # wt = exp(GC*cd^2 + GC*r2): full bilateral weight
                    s.activation(out=sqw[:, :, :, :], in_=sqw[:, :, :, :], func=F.Exp,
                                 scale=GC, bias=bias_r2)
                    for c in range(3):
                        sh_c = imgA[:, c::3, sy : sy + 4, sx : sx + W]
                        v.tensor_tensor(out=pr[:, :, :, :], in0=sqw[:, :, :, :],
                                        in1=sh_c, op=A.mult)
                        v.tensor_tensor(out=num[:, c::3, :, :], in0=num[:, c::3, :, :],
                                        in1=pr[:, :, :, :], op=A.add)
                    g.tensor_tensor(out=den[:, :, :, :], in0=sqw[:, :, :, :],
                                    in1=den[:, :, :, :], op=A.add)
            pimg_cm.__exit__(None, None, None)  # free imgA

            # ---------- divide in place: num becomes filt (f32) ----------
            with tc.tile_pool(name="pdv", bufs=1) as pdv:
                rcp = pdv.tile([128, NB, 4, W], F32, tag="rcp", name="rcp")
                v.reciprocal(out=rcp[:, :, :, :], in_=den[:, :, :, :])
                for c in range(3):
                    v.tensor_tensor(out=num[:, c::3, :, :], in0=num[:, c::3, :, :],
                                    in1=rcp[:, :, :, :], op=A.mult)

            # ---------- NMS: channel-sequential, both images batched ------
            with tc.tile_pool(name="psel", bufs=1, side="right") as psel:
                mags = psel.tile([128, NB, 4, W], F32, tag="mags", name="mags")
                d0m = psel.tile([128, NB, 4, W], U8, tag="d0m", name="d0m")
                d45 = psel.tile([128, NB, 4, W], U8, tag="d45", name="d45")
                d90 = psel.tile([128, NB, 4, W], U8, tag="d90", name="d90")
                pgxy_cm = tc.tile_pool(name="pgxy", bufs=1, side="right")
                pgxy = pgxy_cm.__enter__()
                gxs = pgxy.tile([128, NB, 4, W], F32, tag="gxs", name="gxs")
                gys = pgxy.tile([128, NB, 4, W], F32, tag="gys", name="gys")
                with tc.tile_pool(name="pch", bufs=1, side="right") as pch:
                    tN = pch.tile([128, NB, 1, W], F32, tag="tN", name="tN")
                    bN = pch.tile([128, NB, 1, W], F32, tag="bN", name="bN")
                    sm = pch.tile([128, NB, 6, W], F32, tag="sm", name="sm")
                    gx = pch.tile([128, NB, 4, W], F32, tag="gx", name="gx")
                    gy = pch.tile([128, NB, 4, W], F32, tag="gy", name="gy")
                    mg = pch.tile([128, NB, 4, W], F32, tag="mg", name="mg")
                    sel = d0m  # scratch reuse: d0m is only written in the dir phase

                    def tt(out_, a, b_, op=A.add):
                        v.tensor_tensor(out=out_, in0=a, in1=b_, op=op)

                    for c in range(3):
                        f = num[:, c::3, :, :]
                        nc.sync.dma_start(out=tN[1:128, :, 0, :], in_=f[0:127, :, 3, :])
                        nc.sync.dma_start(out=tN[0:1, :, 0, :], in_=f[0:1, :, 0, :])
                        nc.sync.dma_start(out=bN[0:127, :, 0, :], in_=f[1:128, :, 0, :])
                        nc.sync.dma_start(out=bN[127:128, :, 0, :], in_=f[127:128, :, 3, :])
                        # vertical 1-2-1 -> sm rows 0:4 (syt): 2f + up + down
                        tt(sm[:, :, 1:3, :], f[:, :, 1:3, :], f[:, :, 1:3, :])
                        tt(sm[:, :, 1:3, :], sm[:, :, 1:3, :], f[:, :, 0:2, :])
                        tt(sm[:, :, 1:3, :], sm[:, :, 1:3, :], f[:, :, 2:4, :])
                        tt(sm[:, :, 0:1, :], f[:, :, 0:1, :], f[:, :, 0:1, :])
                        tt(sm[:, :, 0:1, :], sm[:, :, 0:1, :], tN[:, :, 0:1, :])
                        tt(sm[:, :, 0:1, :], sm[:, :, 0:1, :], f[:, :, 1:2, :])
                        tt(sm[:, :, 3:4, :], f[:, :, 3:4, :], f[:, :, 3:4, :])
                        tt(sm[:, :, 3:4, :], sm[:, :, 3:4, :], f[:, :, 2:3, :])
                        tt(sm[:, :, 3:4, :], sm[:, :, 3:4, :], bN[:, :, 0:1, :])
                        syt = sm[:, :, 0:4, :]
                        tt(gx[:, :, :, 1 : W - 1], syt[:, :, :, 2:W], syt[:, :, :, 0 : W - 2], A.subtract)
                        tt(gx[:, :, :, 0:1], syt[:, :, :, 1:2], syt[:, :, :, 0:1], A.subtract)
                        tt(gx[:, :, :, W - 1 : W], syt[:, :, :, W - 1 : W], syt[:, :, :, W - 2 : W - 1], A.subtract)
                        # horizontal 1-2-1 over 6 virtual rows -> sm (sxh);
                        # syt rows consumed above before being overwritten
                        tt(sm[:, :, 1:5, 1 : W - 1], f[:, :, :, 1 : W - 1], f[:, :, :, 1 : W - 1])
                        tt(sm[:, :, 1:5, 1 : W - 1], sm[:, :, 1:5, 1 : W - 1], f[:, :, :, 0 : W - 2])
                        tt(sm[:, :, 1:5, 1 : W - 1], sm[:, :, 1:5, 1 : W - 1], f[:, :, :, 2:W])
                        tt(sm[:, :, 1:5, 0:1], f[:, :, :, 0:1], f[:, :, :, 0:1])
                        tt(sm[:, :, 1:5, 0:1], sm[:, :, 1:5, 0:1], f[:, :, :, 0:1])
                        tt(sm[:, :, 1:5, 0:1], sm[:, :, 1:5, 0:1], f[:, :, :, 1:2])
                        tt(sm[:, :, 1:5, W - 1 : W], f[:, :, :, W - 1 : W], f[:, :, :, W - 1 : W])
                        tt(sm[:, :, 1:5, W - 1 : W], sm[:, :, 1:5, W - 1 : W], f[:, :, :, W - 1 : W])
                        tt(sm[:, :, 1:5, W - 1 : W], sm[:, :, 1:5, W - 1 : W], f[:, :, :, W - 2 : W - 1])
                        for rowdst, rowsrc in ((sm[:, :, 0:1, :], tN[:, :, 0:1, :]),
                                               (sm[:, :, 5:6, :], bN[:, :, 0:1, :])):
                            stt(v, rowdst[:, :, :, 1 : W - 1], rowsrc[:, :, :, 1 : W - 1],
                                rowsrc[:, :, :, 0 : W - 2], A.mult, A.add, 2.0)
                            stt(v, rowdst[:, :, :, 1 : W - 1], rowdst[:, :, :, 1 : W - 1],
                                rowsrc[:, :, :, 2:W], A.add, A.add)
                            stt(v, rowdst[:, :, :, 0:1], rowsrc[:, :, :, 0:1],
                                rowsrc[:, :, :, 1:2], A.mult, A.add, 3.0)
                            stt(v, rowdst[:, :, :, W - 1 : W], rowsrc[:, :, :, W - 1 : W],
                                rowsrc[:, :, :, W - 2 : W - 1], A.mult, A.add, 3.0)
                        tt(gy[:, :, :, :], sm[:, :, 2:6, :], sm[:, :, 0:4, :], A.subtract)
                        ayb = sm[:, :, 0:4, :]  # scratch: sm fully consumed by gy
                        s.activation(out=mg[:, :, :, :], in_=gx[:, :, :, :], func=F.Abs)
                        s.activation(out=ayb, in_=gy[:, :, :, :], func=F.Abs)
                        tt(mg[:, :, :, :], mg[:, :, :, :], ayb)
                        if c == 0:
                            v.tensor_copy(out=gxs[:, :, :, :], in_=gx[:, :, :, :])
                            v.tensor_copy(out=gys[:, :, :, :], in_=gy[:, :, :, :])
                            v.tensor_copy(out=mags[:, :, :, :], in_=mg[:, :, :, :])
                        else:
                            stt(v, sel[:, :, :, :], mg[:, :, :, :], mags[:, :, :, :],
                                A.bypass, A.is_gt)
                            v.copy_predicated(out=gxs[:, :, :, :], mask=sel[:, :, :, :], data=gx[:, :, :, :])
                            v.copy_predicated(out=gys[:, :, :, :], mask=sel[:, :, :, :], data=gy[:, :, :, :])
                            v.copy_predicated(out=mags[:, :, :, :], mask=sel[:, :, :, :], data=mg[:, :, :, :])
                pnum_cm.__exit__(None, None, None)  # free num

                # direction masks
                with tc.tile_pool(name="pdir", bufs=1, side="right") as pdir:
                    u = pdir.tile([128, NB, 4, W], F32, tag="u", name="u")
                    ax = pdir.tile([128, NB, 4, W], F32, tag="ax", name="ax")
                    tA = pdir.tile([128, NB, 4, W], F32, tag="tA", name="tA")
                    TA = pdir.tile([128, NB, 4, W], F32, tag="TA", name="TA")
                    t1 = pdir.tile([128, NB, 4, W], F32, tag="t1", name="t1")
                    b1 = pdir.tile([128, NB, 4, W], U8, tag="b1", name="b1")
                    b2 = pdir.tile([128, NB, 4, W], U8, tag="b2", name="b2")
                    v.tensor_scalar(ax[:, :, :, :], gxs[:, :, :, :], 0.0, 2.0, A.is_ge, A.mult)
                    stt(v, u[:, :, :, :], ax[:, :, :, :], gys[:, :, :, :], A.bypass, A.mult)
                    stt(v, u[:, :, :, :], u[:, :, :, :], gys[:, :, :, :], A.bypass, A.subtract)
                    s.activation(out=ax[:, :, :, :], in_=gxs[:, :, :, :], func=F.Abs)
                    v.tensor_scalar(tA[:, :, :, :], ax[:, :, :, :], T22, None, A.mult)
                    v.tensor_scalar(TA[:, :, :, :], ax[:, :, :, :], T67, None, A.mult)
                    # d0: -tA <= u < tA   (or mag == 0)
                    stt(v, t1[:, :, :, :], u[:, :, :, :], tA[:, :, :, :], A.bypass, A.add)
                    v.tensor_scalar(b1[:, :, :, :], t1[:, :, :, :], 0.0, None, A.is_ge)
                    stt(v, t1[:, :, :, :], u[:, :, :, :], tA[:, :, :, :], A.bypass, A.subtract)
                    v.tensor_scalar(b2[:, :, :, :], t1[:, :, :, :], 0.0, None, A.is_lt)
                    stt(v, d0m[:, :, :, :], b1[:, :, :, :], b2[:, :, :, :], A.bypass, A.logical_and)
                    v.tensor_scalar(b1[:, :, :, :], mags[:, :, :, :], 0.0, None, A.is_equal)
                    stt(v, d0m[:, :, :, :], d0m[:, :, :, :], b1[:, :, :, :], A.bypass, A.logical_or)
                    # d45: tA <= u < TA   (t1 still holds u - tA)
                    v.tensor_scalar(b1[:, :, :, :], t1[:, :, :, :], 0.0, None, A.is_ge)
                    stt(v, t1[:, :, :, :], u[:, :, :, :], TA[:, :, :, :], A.bypass, A.subtract)
                    v.tensor_scalar(b2[:, :, :, :], t1[:, :, :, :], 0.0, None, A.is_lt)
                    stt(v, d45[:, :, :, :], b1[:, :, :, :], b2[:, :, :, :], A.bypass, A.logical_and)
                    # d90: u >= TA or u < -TA   (t1 still holds u - TA)
                    v.tensor_scalar(b1[:, :, :, :], t1[:, :, :, :], 0.0, None, A.is_ge)
                    stt(v, t1[:, :, :, :], u[:, :, :, :], TA[:, :, :, :], A.bypass, A.add)
                    v.tensor_scalar(b2[:, :, :, :], t1[:, :, :, :], 0.0, None, A.is_lt)
                    stt(v, d90[:, :, :, :], b1[:, :, :, :], b2[:, :, :, :], A.bypass, A.logical_or)
                pgxy_cm.__exit__(None, None, None)  # free gxs/gys

                # neighbor picks + suppression (per image: copy_predicated
                # and its neighbor views must stay within 2 free dims)
                with tc.tile_pool(name="pnbr", bufs=1, side="right") as pnbr:
                    mh = pnbr.tile([128, 6, WH], F32, tag="mh", name="mh")
                    n1 = pnbr.tile([128, 4, W], F32, tag="n1", name="n1")
                    n2 = pnbr.tile([128, 4, W], F32, tag="n2", name="n2")
                    for b in range(NB):
                        mb = mags[:, b, :, :]
                        v.memset(mh[:, :, :], 0.0)
                        v.tensor_copy(out=mh[:, 1:5, 1 : 1 + W], in_=mb)
                        nc.sync.dma_start(out=mh[1:128, 0:1, 1 : 1 + W], in_=mb[0:127, 3:4, :])
                        nc.sync.dma_start(out=mh[0:127, 5:6, 1 : 1 + W], in_=mb[1:128, 0:1, :])

                        def nbr(dy2, dx2):
                            return mh[:, 1 + dy2 : 5 + dy2, 1 + dx2 : 1 + dx2 + W]

                        v.tensor_copy(out=n1[:, :, :], in_=nbr(-1, -1))
                        v.copy_predicated(out=n1[:, :, :], mask=d90[:, b, :, :], data=nbr(-1, 0))
                        v.copy_predicated(out=n1[:, :, :], mask=d45[:, b, :, :], data=nbr(-1, 1))
                        v.copy_predicated(out=n1[:, :, :], mask=d0m[:, b, :, :], data=nbr(0, 1))
                        v.tensor_copy(out=n2[:, :, :], in_=nbr(1, 1))
                        v.copy_predicated(out=n2[:, :, :], mask=d90[:, b, :, :], data=nbr(1, 0))
                        v.copy_predicated(out=n2[:, :, :], mask=d45[:, b, :, :], data=nbr(1, -1))
                        v.copy_predicated(out=n2[:, :, :], mask=d0m[:, b, :, :], data=nbr(0, -1))
                        stt(v, n1[:, :, :], mb, n1[:, :, :], A.bypass, A.is_ge)
                        stt(v, n2[:, :, :], mb, n2[:, :, :], A.bypass, A.is_ge)
                        stt(v, n1[:, :, :], n1[:, :, :], n2[:, :, :], A.bypass, A.mult)
                        stt(v, nmst[:, b, :, :], mb, n1[:, :, :], A.bypass, A.mult)

            # ---------- hysteresis (both images batched) ----------
            with tc.tile_pool(name="phy", bufs=1) as phy:
                st = phy.tile([128, NB, 4, WH], F16, tag="st", name="st")
                sc = phy.tile([128, NB, 4, WH], F16, tag="sc", name="sc")
                wk = phy.tile([128, NB, 4, WH], F16, tag="wk", name="wk")
                hdil = phy.tile([128, NB, 4, WH], F16, tag="hdil", name="hdil")
                vdil = phy.tile([128, NB, 6, WH], F16, tag="vdil", name="vdil")
                dil = phy.tile([128, NB, 4, WH], F16, tag="dil", name="dil")
                v.memset(st[:, :, :, :], 0.0)
                v.memset(wk[:, :, :, :], 0.0)
                v.memset(hdil[:, :, :, :], 0.0)
                v.memset(vdil[:, :, :, :], 0.0)
                thr = phy.tile([128, NB, 4, W], F16, tag="thr", name="thr")
                v.tensor_scalar(thr[:, :, :, :], nmst[:, :, :, :], HIGH_T, None, A.is_gt)
                nc.sync.dma_start(out=st[:, :, :, 1 : 1 + W], in_=thr[:, :, :, :])
                v.tensor_scalar(thr[:, :, :, :], nmst[:, :, :, :], LOW_T, None, A.is_gt)
                nc.sync.dma_start(out=wk[:, :, :, 1 : 1 + W], in_=thr[:, :, :, :])
                st2d = st[:, :, :, :].rearrange("p i a x -> p (i a x)")
                sc2d = sc[:, :, :, :].rearrange("p i a x -> p (i a x)")
                wk2d = wk[:, :, :, :].rearrange("p i a x -> p (i a x)")
                for _it in range(HYST_ITERS):
                    v.tensor_tensor_scan(out=sc2d, data0=wk2d, data1=st2d,
                                         initial=0.0, op0=A.mult, op1=A.max)
                    v.tensor_tensor_scan(out=st2d[:, ::-1], data0=wk2d[:, ::-1],
                                         data1=sc2d[:, ::-1], initial=0.0, op0=A.mult, op1=A.max)
                    v.tensor_tensor(out=hdil[:, :, :, 1 : 1 + W], in0=st[:, :, :, 0:W],
                                    in1=st[:, :, :, 2 : 2 + W], op=A.max)
                    v.tensor_tensor(out=vdil[:, :, 1:5, 1 : 1 + W], in0=hdil[:, :, :, 1 : 1 + W],
                                    in1=st[:, :, :, 1 : 1 + W], op=A.max)
                    nc.sync.dma_start(out=vdil[1:128, :, 0:1, 1 : 1 + W], in_=vdil[0:127, :, 4:5, 1 : 1 + W])
                    nc.sync.dma_start(out=vdil[0:127, :, 5:6, 1 : 1 + W], in_=vdil[1:128, :, 1:2, 1 : 1 + W])
                    stt(v, dil[:, :, :, :], vdil[:, :, 0:4, :], vdil[:, :, 2:6, :], A.bypass, A.max)
                    stt(v, dil[:, :, :, :], dil[:, :, :, :], vdil[:, :, 1:5, :], A.bypass, A.max)
                    stt(v, dil[:, :, :, :], dil[:, :, :, :], wk[:, :, :, :], A.bypass, A.mult)
                    stt(v, st[:, :, :, :], st[:, :, :, :], dil[:, :, :, :], A.bypass, A.max)
                    # (all operands above are contiguous or row-range views that
                    # canonically merge to <=2 free dims)
                o32 = phy.tile([128, NB, 4, W], F32, tag="o32", name="o32")
                v.tensor_tensor(out=o32[:, :, :, :], in0=st[:, :, :, 1 : 1 + W],
                                in1=st[:, :, :, 1 : 1 + W], op=A.max)
                nc.sync.dma_start(
                    out=out[:, :, :].rearrange("b (p r) x -> p b r x", r=4),
                    in_=o32[:, :, :, :],
                )
            pacc_cm.__exit__(None, None, None)
    nc.finalize()
    return nc


# ---------------------------------------------------------------------------
# walrus 1-sync-wait-per-instruction workaround (BIR JSON post-pass)
# ---------------------------------------------------------------------------
import json as _json

_ws_counter = [0]


def _split_instruction_list(instrs):
    out = []
    for ins in instrs:
        si = ins.get("sync_info")
        waits = (si or {}).get("on_wait") or []
        if len(waits) > 1:
            for wcond in waits[:-1]:
                _ws_counter[0] += 1
                out.append({
                    "debug": ins.get("debug", 0),
                    "engine": ins["engine"],
                    "ins": [],
                    "name": f"I-waitsplit-{_ws_counter[0]}",
                    "opcode": "NoOp",
                    "outs": [],
                    "sync_info": {"on_wait": [wcond], "on_update": []},
                })
            si = dict(si)
            si["on_wait"] = [waits[-1]]
            ins = dict(ins)
            ins["sync_info"] = si
        out.append(ins)
    return out


def _walk_split(obj):
    if isinstance(obj, dict):
        for k, val in obj.items():
            if k == "instructions" and isinstance(val, list):
                obj[k] = _split_instruction_list(val)
            else:
                _walk_split(val)
    elif isinstance(obj, list):
        for val in obj:
            _walk_split(val)


def _split_multiwait_bir(bir_json):
    j = _json.loads(bir_json)
    _walk_split(j)
    return _json.dumps(j).encode()


_patched = [False]


def _install_bir_patch():
    if _patched[0]:
        return
    _patched[0] = True
    import concourse.bass_utils as bu

    orig = bu.compile_bir_kernel

    def patched(bir_json, tmpdir, neff_name="file.neff"):
        return orig(_split_multiwait_bir(bir_json), tmpdir, neff_name)

    bu.compile_bir_kernel = patched
    try:
        import concourse.bass2jax as b2j

        b2j.compile_bir_kernel = patched
    except Exception:
        pass


# ---------------------------------------------------------------------------
# host entry point
# ---------------------------------------------------------------------------
_cache = {}


def _get_program(rep=1):
    key = ("nc", rep)
    if key not in _cache:
        _install_bir_patch()
        _cache[key] = build(rep=rep)
    return _cache[key]


def make_in_maps(x):
    x = np.asarray(x, dtype=np.float32)
    xpad = np.pad(x, ((0, 0), (0, 0), (PAD, PAD), (PAD, PAD)), mode="reflect")
    return [
        {"xp": np.ascontiguousarray(xpad[i * NB : (i + 1) * NB])}
        for i in range(NCORES)
    ]


def kernel(x):
    """x: [16,3,512,512] float32 -> edges [16,1,512,512] float32."""
    from concourse.bass_utils import run_bass_kernel_spmd

    x = np.asarray(x, dtype=np.float32)
    B = x.shape[0]
    assert x.shape == (NCORES * NB, 3, H, W), x.shape
    nc = _get_program()
    in_maps = make_in_maps(x)
    res = run_bass_kernel_spmd(nc, in_maps, core_ids=list(range(NCORES)))
    out = np.empty((B, 1, H, W), np.float32)
    for i in range(NCORES):
        out[i * NB : (i + 1) * NB, 0] = res.results[i]["edges"]
    return out
